# revision 59
# baseline (speedup 1.0000x reference)
"""Multi-head self-attention (B=4, S=1024, D=1024, H=16, RoPE, causal) on 8
Trainium2 NeuronCores.

Sharding: 8 cores = 4 batches x 2 head-groups (8 heads each). Each core
computes QKV projections for its batch/head-group, RoPE, causal attention,
and a partial output projection (contraction over its 512 attention dims).
The host sums the two partial outputs per batch (the "all-reduce") and
concatenates batches.

Design:
- All matmul operands bf16 (1 cyc/row at any tile size; half the DMA).
- Q/K are projected TRANSPOSED (weights stationary, x streamed): no PE
  transposes. RoPE runs in [dg, s] layout; the even/odd partner rows come
  from a PE permutation matmul (psw) emitted one tile late so its qs
  dependency never stalls the PE; then 2 DVE multiplies + 1 DVE add.
- Causal attention splits into two q-halves. Half-0 (q<512, k-tiles 0..3)
  interleaves with the second half of the projections so ACT exp time
  hides under PE projection time. The transposed-logits layout (L^T[k,q])
  gives softmax sums via a ones column appended to V.
- Softmax 1/sum per head pair: both s-rows batched through one direct
  SBUF->SBUF reshape DMA to [128,8], one reciprocal, one DRAM
  partition-broadcast read, one DVE multiply per head; odd heads write
  partitions 64:128 directly (cross-partition-offset engine writes work
  at 32-partition granularity). The kernel-tail chains use the ACT hwdge
  queue, which is idle by then.
- tril masking on GpSimd; exp on ACT; the half-0 output projection
  interleaves with half-1 attention; the last attention pair runs its two
  heads sequentially so one softmax chain hides under the other head; the
  tail output projection runs on a deep 4-bank PSUM ring. y is written
  bf16 (host sums the two partial outputs in f32).
"""

import numpy as np

import concourse.bass as bass
import concourse.mybir as mybir
import concourse.tile as tile
from concourse.bass import ts
from concourse.bass_utils import run_bass_kernel_spmd
from concourse.masks import make_upper_triangular

B, S, D = 4, 1024, 1024
H = 16  # total heads
HG = 8  # heads per core (head-group)
DK = 64  # head dim
DG = HG * DK  # 512, per-core projection width
ROPE_THETA = 10000.0
P = 128  # partitions
NS = S // P  # 8 s-tiles
ND = D // P  # 8 d-chunks
F32 = mybir.dt.float32
BF16 = mybir.dt.bfloat16
EXP = mybir.ActivationFunctionType.Exp

_uid = [0]


def _split_excess_waits(nc, limit=1):
    """This container's walrus rejects >1 sync waits on the kernel-tail
    Drain; move excess waits onto same-engine NoOps inserted before it."""
    for f in nc.m.functions:
        for blk in f.blocks:
            insts = list(blk.instructions)
            out = []
            changed = False
            for inst in insts:
                si = inst.sync_info
                if si is not None and si.on_wait and len(si.on_wait) > limit:
                    waits = list(si.on_wait)
                    head, tail = waits[:-limit], waits[-limit:]
                    for i in range(0, len(head), limit):
                        _uid[0] += 1
                        nop = mybir.InstNoOp(
                            name=f"waitsplit-{_uid[0]}", ins=[], outs=[]
                        )
                        nop.engine = inst.engine
                        nop.sync_info = mybir.SyncInfo(
                            on_wait=head[i : i + limit], on_update=[]
                        )
                        out.append(nop)
                    si.on_wait = tail
                    changed = True
                out.append(inst)
            if changed:
                blk.instructions = out
    return nc


def build_nc():
    nc = bass.Bass("TRN2")
    xT = nc.dram_tensor("xT", [D, S], BF16, kind="ExternalInput")
    wqT = nc.dram_tensor("wqT", [D, DG], BF16, kind="ExternalInput")
    wkT = nc.dram_tensor("wkT", [D, DG], BF16, kind="ExternalInput")
    wvT = nc.dram_tensor("wvT", [D, DG], BF16, kind="ExternalInput")
    woT = nc.dram_tensor("woT", [DG, D], BF16, kind="ExternalInput")
    cosT = nc.dram_tensor("cosT", [P, S], BF16, kind="ExternalInput")
    sinTs = nc.dram_tensor("sinTs", [P, S], BF16, kind="ExternalInput")
    pswT = nc.dram_tensor("pswT", [P, P], BF16, kind="ExternalInput")
    yT = nc.dram_tensor("yT", [D, S], BF16, kind="ExternalOutput")
    # DRAM scratch row for the softmax 1/sum partition-broadcast
    rbcd = nc.dram_tensor("rbcd", [H, 1024], F32)

    with tile.TileContext(nc) as tc:
        with (
            tc.tile_pool(name="const", bufs=1) as constp,
            tc.tile_pool(name="wts", bufs=1) as wp,
            tc.tile_pool(name="big", bufs=1) as bigp,
            tc.tile_pool(name="qsw", bufs=4) as qswp,
            tc.tile_pool(name="rr", bufs=2) as rrp,
            tc.tile_pool(name="ysb", bufs=3) as ysp,
        ):
            # ---- constants ----
            ztrilf = constp.tile([P, P], F32, tag="ztrilf")
            nc.vector.memset(ztrilf[:, :], 0.0)
            make_upper_triangular(nc, ztrilf[:, :], val=1.0, diag=True)
            ztril = constp.tile([P, P], BF16, tag="ztril")
            nc.vector.tensor_copy(ztril[:, :], ztrilf[:, :])

            # ---- resident weights/activations ----
            xs = wp.tile([P, ND, S], BF16, tag="xs", name="xs")
            wq_all = wp.tile([P, ND, DG], BF16, tag="wq", name="wq")
            wk_all = wp.tile([P, ND, DG], BF16, tag="wk", name="wk")
            wv_all = wp.tile([P, ND, DG], BF16, tag="wv", name="wv")
            wo_all = wp.tile([P, DG // P, D], BF16, tag="wo", name="wo")
            cs = wp.tile([P, S], BF16, tag="cs", name="cs")
            sn = wp.tile([P, S], BF16, tag="sn", name="sn")
            psw = wp.tile([P, P], BF16, tag="psw", name="psw")

            for c in range(ND):
                nc.sync.dma_start(out=xs[:, c, :], in_=xT[ts(c, P), :])
            # wv per-chunk: V0's chunk-c matmul unblocks as chunk c lands
            for c in range(ND):
                nc.scalar.dma_start(out=wv_all[:, c, :], in_=wvT[ts(c, P), :])
            nc.scalar.dma_start(out=psw[:, :], in_=pswT[:, :])
            nc.scalar.dma_start(out=cs[:, :], in_=cosT[:, :])
            nc.scalar.dma_start(out=sn[:, :], in_=sinTs[:, :])
            nc.sync.dma_start(
                out=wq_all[:, :, :],
                in_=wqT[:, :].rearrange("(c p) o -> p c o", p=P),
            )
            nc.sync.dma_start(
                out=wk_all[:, :, :],
                in_=wkT[:, :].rearrange("(c p) o -> p c o", p=P),
            )
            nc.sync.dma_start(
                out=wo_all[:, :, :],
                in_=woT[:, :].rearrange("(c p) o -> p c o", p=P),
            )
            # dummy exp AFTER the DMA triggers: pulls the ACT table load
            # (~2.7us) into the startup shadow without delaying the queues
            dummy = constp.tile([1, 8], F32, tag="dummy")
            nc.scalar.activation(
                out=dummy[:, :], in_=ztrilf[0:1, 0:8], func=EXP, scale=1.0
            )

            # persistent: q^T/k^T pair tiles [128 dims, S], v tiles, at tiles
            qt_sb = [bigp.tile([P, S], BF16, tag=f"qt{p}", name=f"qt{p}") for p in range(4)]
            kt_sb = [bigp.tile([P, S], BF16, tag=f"kt{p}", name=f"kt{p}") for p in range(4)]
            v_sb = [bigp.tile([P, HG, DK + 1], BF16, tag=f"v{j}", name=f"v{j}") for j in range(NS)]
            at_sb = [bigp.tile([P, S], BF16, tag=f"at{p}", name=f"at{p}") for p in range(4)]
            for j in range(NS):
                nc.vector.memset(v_sb[j][:, :, DK : DK + 1], 1.0)

            def normalize(heads, hx, dq=None):
                # softmax 1/sum for 1 or 2 heads through one DMA chain:
                # s-rows -> [128,w] reshape (direct SBUF DMA), reciprocal,
                # DRAM round trip for the partition-broadcast, multiply
                dq = dq or nc.sync
                nh = len(heads)
                w = 4 * nh
                sr2 = rrp.tile([P, 512], F32, tag="sr", name="sr2")
                for i, (h, ap) in enumerate(heads):
                    nc.vector.tensor_copy(
                        sr2[32 * i : 32 * i + 1, :], ap[DK : DK + 1, 0:512]
                    )
                slot = heads[0][0] + (HG if hx else 0)
                src = bass.AP(
                    tensor=sr2[:, :].tensor,
                    offset=sr2[:, :].offset,
                    ap=[[32 * 512, nh], [1, 512]],
                )
                rs = rrp.tile([P, 8], F32, tag="rs", name="rs")
                dq.dma_start(out=rs[:, 0:w], in_=src)
                rc = rrp.tile([P, 8], F32, tag="rc", name="rc")
                nc.vector.reciprocal(out=rc[:, 0:w], in_=rs[:, 0:w])
                dq.dma_start(
                    out=rbcd[slot, 0 : 512 * nh].rearrange("(p c) -> p c", c=w),
                    in_=rc[:, 0:w],
                )
                row = rbcd[slot, :]
                bc_src = bass.AP(
                    tensor=row.tensor, offset=row.offset, ap=[[0, DK], [1, 512 * nh]]
                )
                rbc2 = rrp.tile([DK, 1024], F32, tag="rbc", name="rbc2")
                dq.dma_start(out=rbc2[:, 0 : 512 * nh], in_=bc_src)
                for i, (h, ap) in enumerate(heads):
                    pair, poff = h // 2, 64 * (h % 2)
                    nc.vector.tensor_mul(
                        at_sb[pair][poff : poff + DK, hx : hx + 512],
                        ap[0:DK, 0:512],
                        rbc2[:, 512 * i : 512 * i + 512],
                    )

            with (
                tc.tile_pool(name="projv", bufs=3, space="PSUM") as pvp,
                tc.tile_pool(name="lg0", bufs=2, space="PSUM") as lg0p,
                tc.tile_pool(name="ap0", bufs=3, space="PSUM") as ap0p,
                tc.tile_pool(name="pt0", bufs=5) as pt0p,
            ):
                def emit_v(i):
                    vp = pvp.tile([P, DG], F32, tag="pv", name=f"v{i}")
                    for c in range(ND):
                        nc.tensor.matmul(
                            vp[:, :], lhsT=xs[:, c, ts(i, P)], rhs=wv_all[:, c, :],
                            start=(c == 0), stop=(c == ND - 1),
                        )
                    nc.vector.tensor_copy(
                        v_sb[i][:, :, 0:DK],
                        vp[:, :].rearrange("p (h c) -> p h c", h=HG),
                    )

                # two-stage software pipeline for transposed projections:
                # emit tile n's proj matmuls, then tile n-1's psw matmul
                # (whose qs copy is ready by then) and rope
                rope_pend = []

                def rope_flush():
                    while rope_pend:
                        rope_pend.pop(0)()

                def emit_qkt(src, pair, hx):
                    w_all = wq_all if src == "q" else wk_all
                    dst = qt_sb[pair] if src == "q" else kt_sb[pair]
                    pp = pvp.tile([P, DG], F32, tag="pv", name=f"{src}{pair}_{hx}")
                    for c in range(ND):
                        nc.tensor.matmul(
                            pp[:, :], lhsT=w_all[:, c, ts(pair, P)],
                            rhs=xs[:, c, hx : hx + 512],
                            start=(c == 0), stop=(c == ND - 1),
                        )
                    qs = qswp.tile([P, 512], BF16, tag="qs", name="qs")
                    nc.scalar.copy(out=qs[:, :], in_=pp[:, :])

                    def finish():
                        qw = pvp.tile([P, DG], F32, tag="pv", name="qw")
                        nc.tensor.matmul(
                            qw[:, :], lhsT=psw[:, :], rhs=qs[:, :],
                            start=True, stop=True,
                        )
                        t1 = qswp.tile([P, 512], BF16, tag="t1", name="t1")
                        nc.vector.tensor_mul(t1[:, :], qs[:, :], cs[:, hx : hx + 512])
                        t2 = qswp.tile([P, 512], BF16, tag="t2", name="t2")
                        nc.vector.tensor_mul(t2[:, :], qw[:, :], sn[:, hx : hx + 512])
                        nc.vector.tensor_add(dst[:, hx : hx + 512], t1[:, :], t2[:, :])

                    rope_pend.append(finish)
                    if len(rope_pend) > 1:
                        rope_pend.pop(0)()

                def ev0(h, ap, j, q0, n, pt):
                    nc.tensor.matmul(
                        ap[0 : DK + 1, q0:512],
                        lhsT=v_sb[j][:, h, :], rhs=pt[:, 0:n],
                        start=(j == 0), stop=(j == 3),
                        skip_group_check=True,
                    )

                def emit_h0_head(h, extra=None):
                    # half-0: q in [0,512), k-tiles 0..3
                    pair, poff = h // 2, 64 * (h % 2)
                    ap = ap0p.tile([P, 512], F32, tag="ap", name=f"ap0_{h}")
                    pend = []
                    for j in range(4):
                        q0 = 128 * j
                        n = 512 - q0
                        lg = lg0p.tile([P, 512], F32, tag="lg", name="lg0")
                        nc.tensor.matmul(
                            lg[:, 0:n],
                            lhsT=kt_sb[pair][poff : poff + DK, ts(j, P)],
                            rhs=qt_sb[pair][poff : poff + DK, q0:512],
                            start=True, stop=True,
                        )
                        pt = pt0p.tile([P, 512], BF16, tag="pt", name="pt0")
                        nc.scalar.activation(
                            out=pt[:, 0:n], in_=lg[:, 0:n], func=EXP, scale=0.125
                        )
                        nc.gpsimd.tensor_mul(pt[:, 0:P], pt[:, 0:P], ztril[:, :])
                        pend.append((j, q0, n, pt))
                        while len(pend) > 2:
                            ev0(h, ap, *pend.pop(0))
                    if extra is not None:
                        extra()  # PE filler while the tail exp/tril drain
                    for args in pend:
                        ev0(h, ap, *args)
                    return ap

                # ---- emission: phase A half-0, half-0 attn, phase A half-1
                for i in range(4):
                    emit_v(i)
                aps0 = {}
                for pair in range(4):
                    emit_qkt("q", pair, 0)
                    emit_qkt("k", pair, 0)
                    if pair == 1:
                        aps0[1] = emit_h0_head(1)
                    elif pair == 2:
                        aps0[0] = emit_h0_head(0)
                        normalize([(1, aps0.pop(1)), (0, aps0.pop(0))], 0)
                    elif pair == 3:
                        aps0[3] = emit_h0_head(3)
                emit_v(4)
                aps0[2] = emit_h0_head(2)
                normalize([(3, aps0.pop(3)), (2, aps0.pop(2))], 0)
                emit_v(5)
                aps0[5] = emit_h0_head(5, extra=lambda: (
                    emit_qkt("q", 0, 512), emit_qkt("k", 0, 512)))
                emit_v(6)
                aps0[4] = emit_h0_head(4, extra=lambda: (
                    emit_qkt("q", 1, 512), emit_qkt("k", 1, 512)))
                normalize([(5, aps0.pop(5)), (4, aps0.pop(4))], 0)
                emit_v(7)
                aps0[7] = emit_h0_head(7, extra=lambda: (
                    emit_qkt("q", 2, 512), emit_qkt("k", 2, 512)))
                aps0[6] = emit_h0_head(6, extra=lambda: (
                    emit_qkt("q", 3, 512), emit_qkt("k", 3, 512)))
                rope_flush()
                normalize([(7, aps0.pop(7)), (6, aps0.pop(6))], 0)

            # ---- half-1 attention + output projection ----
            with (
                tc.tile_pool(name="ypt", bufs=1, space="PSUM") as yptp,
                tc.tile_pool(name="lg1", bufs=3, space="PSUM") as lg1p,
                tc.tile_pool(name="ap1", bufs=4, space="PSUM") as ap1p,
                tc.tile_pool(name="pt1", bufs=12) as pt1p,
            ):
                def h1_qk(h, j):
                    # one k-tile of half-1 QK + exp (+ tril for diag tiles)
                    pair, poff = h // 2, 64 * (h % 2)
                    lo = max(512, 128 * j)
                    n = 1024 - lo
                    lg = lg1p.tile([P, 512], F32, tag="lg", name="lg1")
                    nc.tensor.matmul(
                        lg[:, 0:n],
                        lhsT=kt_sb[pair][poff : poff + DK, ts(j, P)],
                        rhs=qt_sb[pair][poff : poff + DK, lo:1024],
                        start=True, stop=True,
                    )
                    pt = pt1p.tile([P, 512], BF16, tag="pt", name="pt1")
                    nc.scalar.activation(
                        out=pt[:, 0:n], in_=lg[:, 0:n], func=EXP, scale=0.125
                    )
                    if 128 * j >= 512:  # diagonal block leads this tile
                        nc.gpsimd.tensor_mul(pt[:, 0:P], pt[:, 0:P], ztril[:, :])
                    return (j, lo, n, pt)

                def ev1(h, ap, j, lo, n, pt):
                    nc.tensor.matmul(
                        ap[0 : DK + 1, lo - 512 : 512],
                        lhsT=v_sb[j][:, h, :], rhs=pt[:, 0:n],
                        start=(j == 0), stop=(j == NS - 1),
                        skip_group_check=True,
                    )

                def emit_outproj(o, hx, copy_eng):
                    ypt = yptp.tile([P, 512], F32, tag="y", name=f"y{o}_{hx}")
                    for c in range(DG // P):
                        nc.tensor.matmul(
                            ypt[:, :],
                            lhsT=wo_all[:, c, ts(o, P)],
                            rhs=at_sb[c][:, hx : hx + 512],
                            start=(c == 0), stop=(c == DG // P - 1),
                        )
                    ysb = ysp.tile([P, 512], BF16, tag="ysb", name="ysb")
                    if copy_eng == "act":
                        nc.scalar.copy(out=ysb[:, :], in_=ypt[:, :])
                    else:
                        nc.vector.tensor_copy(ysb[:, :], ypt[:, :])
                    nc.sync.dma_start(out=yT[ts(o, P), hx : hx + 512], in_=ysb[:, :])

                # first four heads (chunks 0,1) interleaved round-robin per
                # k-tile: 4 independent QK->exp->EV streams hide the chain
                # latency and keep the PE clock up
                g1 = (1, 0, 3, 2)
                aps = {h: ap1p.tile([P, 512], F32, tag="ap", name=f"ap1_{h}")
                       for h in g1}
                pend = []
                for j in range(NS):
                    for h in g1:
                        pend.append((h, aps[h]) + h1_qk(h, j))
                    if j % 2 == 1:
                        emit_outproj(j // 2, 0, "act")
                    while len(pend) > 8:
                        a = pend.pop(0)
                        ev1(a[0], a[1], *a[2:])
                for a in pend:
                    ev1(a[0], a[1], *a[2:])
                normalize([(1, aps[1]), (0, aps[0])], 512)
                normalize([(3, aps[3]), (2, aps[2])], 512)

                emit_h1_pair_extra = lambda: (
                    emit_outproj(4, 0, "act"), emit_outproj(5, 0, "act"))
                apa = ap1p.tile([P, 512], F32, tag="ap", name="ap1_5")
                apb = ap1p.tile([P, 512], F32, tag="ap", name="ap1_4")
                pend = []
                for j in range(NS):
                    pend.append((5, apa) + h1_qk(5, j))
                    pend.append((4, apb) + h1_qk(4, j))
                    while len(pend) > 4:
                        a = pend.pop(0)
                        ev1(a[0], a[1], *a[2:])
                emit_h1_pair_extra()
                for a in pend:
                    ev1(a[0], a[1], *a[2:])
                normalize([(5, apa), (4, apb)], 512)

                # last pair: heads sequential with immediate per-head
                # normalize so the tail softmax chain overlaps attention
                apa = ap1p.tile([P, 512], F32, tag="ap", name="ap1_7")
                pend = []
                for j in range(NS):
                    pend.append(h1_qk(7, j))
                    if j == 2:
                        emit_outproj(6, 0, "act")
                    while len(pend) > 2:
                        ev1(7, apa, *pend.pop(0))
                for a in pend:
                    ev1(7, apa, *a)
                normalize([(7, apa)], 512, nc.scalar)
                apb = ap1p.tile([P, 512], F32, tag="ap", name="ap1_6")
                pend = []
                for j in range(NS):
                    pend.append(h1_qk(6, j))
                    if j == 2:
                        emit_outproj(7, 0, "act")
                    while len(pend) > 2:
                        ev1(6, apb, *pend.pop(0))
                for a in pend:
                    ev1(6, apb, *a)
                normalize([(6, apb)], 512, nc.scalar)

            # tail output projection: the first four o-tiles accumulate
            # their c=0..2 partials up front (independent of the final
            # normalize, so the PE chews them while that chain drains),
            # then take their c=3 finals; the rest stream on the ring
            with tc.tile_pool(name="ypt2", bufs=4, space="PSUM") as ypt2p:
                def yfin(o, ypt):
                    ysb = ysp.tile([P, 512], BF16, tag="ysb", name="ysb")
                    if o % 2:
                        nc.scalar.copy(out=ysb[:, :], in_=ypt[:, :])
                    else:
                        nc.vector.tensor_copy(ysb[:, :], ypt[:, :])
                    nc.sync.dma_start(out=yT[ts(o, P), 512:1024], in_=ysb[:, :])

                yts = []
                for o in range(4):
                    ypt = ypt2p.tile([P, 512], F32, tag="y", name=f"yt{o}")
                    for c in range(3):
                        nc.tensor.matmul(
                            ypt[:, :], lhsT=wo_all[:, c, ts(o, P)],
                            rhs=at_sb[c][:, 512:1024],
                            start=(c == 0), stop=False,
                        )
                    yts.append(ypt)
                for o, ypt in enumerate(yts):
                    nc.tensor.matmul(
                        ypt[:, :], lhsT=wo_all[:, 3, ts(o, P)],
                        rhs=at_sb[3][:, 512:1024],
                        start=False, stop=True,
                    )
                    yfin(o, ypt)
                for o in range(4, ND):
                    ypt = ypt2p.tile([P, 512], F32, tag="y", name=f"yt{o}")
                    for c in range(DG // P):
                        nc.tensor.matmul(
                            ypt[:, :], lhsT=wo_all[:, c, ts(o, P)],
                            rhs=at_sb[c][:, 512:1024],
                            start=(c == 0), stop=(c == DG // P - 1),
                        )
                    yfin(o, ypt)

    _split_excess_waits(nc)
    return nc


_NC_CACHE = {}


def _get_nc():
    if "nc" not in _NC_CACHE:
        _NC_CACHE["nc"] = build_nc()
    return _NC_CACHE["nc"]


# rotate-half permutation within each head: evens then odds
_PERM = np.concatenate([np.arange(0, DK, 2), np.arange(1, DK, 2)])


def _bf16(a):
    import ml_dtypes

    return np.asarray(a, dtype=ml_dtypes.bfloat16)


def _host_prep(x, Wq, Wk, Wv, Wo, token_positions):
    """Build the 8 per-core input dicts."""
    inv_freq = 1.0 / (ROPE_THETA ** (np.arange(0, DK, 2, dtype=np.float32) / DK))
    in_maps = []
    for core in range(8):
        b, g = core // 2, core % 2
        heads = np.arange(HG * g, HG * (g + 1))
        rows_qk = (heads[:, None] * DK + _PERM[None, :]).reshape(-1)
        rows_v = (heads[:, None] * DK + np.arange(DK)[None, :]).reshape(-1)
        pos = token_positions[b].astype(np.float32)  # [S]
        ang = pos[None, :] * inv_freq[:, None]  # [32, S]
        cosT = np.tile(np.cos(ang), (4, 1)).astype(np.float32)  # [128, S]
        sin = np.sin(ang)
        sinTs = np.concatenate([-sin, sin, -sin, sin], axis=0).astype(np.float32)
        psw = np.zeros((P, P), dtype=np.float32)
        psw[np.arange(P) ^ 32, np.arange(P)] = 1.0
        in_maps.append(
            {
                "xT": _bf16(x[b].T),
                "wqT": _bf16(Wq[rows_qk, :].T),
                "wkT": _bf16(Wk[rows_qk, :].T),
                "wvT": _bf16(Wv[rows_v, :].T),
                "woT": _bf16(Wo[:, rows_v].T),
                "cosT": _bf16(cosT),
                "sinTs": _bf16(sinTs),
                "pswT": _bf16(psw),
            }
        )
    return in_maps


def kernel(x, Wq, Wk, Wv, Wo, token_positions, _trace=False):
    x = np.asarray(x, dtype=np.float32)
    Wq = np.asarray(Wq, dtype=np.float32)
    Wk = np.asarray(Wk, dtype=np.float32)
    Wv = np.asarray(Wv, dtype=np.float32)
    Wo = np.asarray(Wo, dtype=np.float32)
    token_positions = np.asarray(token_positions)

    nc = _get_nc()
    in_maps = _host_prep(x, Wq, Wk, Wv, Wo, token_positions)
    res = run_bass_kernel_spmd(nc, in_maps, core_ids=list(range(8)), trace=_trace)
    if _trace:
        kernel.last_exec_time_ns = res.exec_time_ns
        kernel.last_results = res

    y = np.empty((B, S, D), dtype=np.float32)
    for b in range(B):
        yT0 = np.asarray(res.results[2 * b]["yT"], dtype=np.float32)
        yT1 = np.asarray(res.results[2 * b + 1]["yT"], dtype=np.float32)
        y[b] = (yT0 + yT1).T
    return y
